# revision 28
# baseline (speedup 1.0000x reference)
"""Single-head encoder attention block on 8 Trainium2 NeuronCores.

Math (per batch element b):
    q = x @ wq.T ; k = x @ wk.T ; v = x @ wv.T
    scores = (q @ k.T) / sqrt(1024) ; attn = softmax(scores, -1)
    out = (attn @ v) @ wo.T

Sharding: data-parallel over batch — batch 8 maps 1:1 onto the 8 cores;
no collectives.

Weight folding (host, one-time input transformation):
    m  = (wq.T @ wk) / 32          scores   = x m x.T
    ut = (wo @ wv).T               out      = attn @ x @ ut
m, ut are cast to bf16 on host and loaded in natural layout (no device
transposes needed for weights).

Per-core device algorithm (bf16 matmul operands, fp32 PSUM):
  A: xT loaded directly (host uploads x pre-transposed bf16),
     Z = x@ut, then F = (x m).T. xT, Z, F all SBUF-resident (no spills).
  B: per i-superblock of 512:
     scoresT[j,i] = xT.T F ; e = exp(scoresT)          (ACT)
     Tacc = sum_jtiles e                               (DVE)
     R = partition_all_reduce(Tacc)                    (GPSIMD)
     out[i,do] = sum_j e[j,i-tile] Z[j,do]             (PE)
     rc = 1/R via tiny PE transposes + DVE recip; out evict scaled by rc.
"""

import os
import sys

for _p in ("/opt/trn_rl_repo", "/root/.axon_site/_ro/trn_rl_repo"):
    if os.path.isdir(_p) and _p not in sys.path:
        sys.path.insert(0, _p)

import numpy as np
from contextlib import ExitStack

import concourse.bacc as bacc
import concourse.tile as tile
from concourse import mybir, masks
from concourse.bass import bass_isa
from concourse.bass_utils import run_bass_kernel_spmd

P = 128
S = 2048          # sequence length (per core)
D = 1024          # model dim = dk = dv
NS = S // P       # 16 seq tiles
ND = D // P       # 8 dim tiles
SB = 512          # i-superblock width (query columns per block)
NSB = S // SB     # 4 superblocks
NIT = SB // P     # 4 i-tiles per superblock
N_CORES = 8

F32 = mybir.dt.float32
BF = mybir.dt.bfloat16
EXP = mybir.ActivationFunctionType.Exp
COPY = mybir.ActivationFunctionType.Copy


def _build():
    nc = bacc.Bacc("TRN2", target_bir_lowering=False, debug=False, num_devices=N_CORES)

    # all inputs host-packed to match SBUF layouts 1:1 (few big DMAs)
    xt_in = nc.dram_tensor("xt", [P, ND * S], BF, kind="ExternalInput").ap()
    m_in = nc.dram_tensor("m", [P, ND * D], BF, kind="ExternalInput").ap()
    u_in = nc.dram_tensor("ut", [P, ND * D], BF, kind="ExternalInput").ap()
    out_d = nc.dram_tensor("out", [S, D], F32, kind="ExternalOutput").ap()

    mm = nc.tensor.matmul

    with tile.TileContext(nc) as tc, ExitStack() as top:
        res = top.enter_context(tc.tile_pool(name="res", bufs=1))
        ident_f32 = res.tile([P, P], F32)
        masks.make_identity(nc, ident_f32[:])
        # xT chunk-major: column c*4096 + d*512 + (s%512), c = s//512. Every
        # matmul slice (width 128 or 512) stays inside one 512-wide s-chunk.
        xt = res.tile([P, ND * S], BF)

        def xts(d, s0, w):
            c, o = divmod(s0, 512)
            base = c * (ND * 512) + d * 512 + o
            return xt[:, base: base + w]
        zres = res.tile([P, NS * D], BF)  # Z: j-tile -> [:, j*D:(j+1)*D] = [j-part, do]
        fres = res.tile([P, ND * S], BF)  # F: d2-tile -> [:, t*S:(t+1)*S] = [d2-part, i]
        un = res.tile([P, ND * D], BF)    # ut natural: d-tile -> [:, t*D:(t+1)*D] = [d-part, do]
        mn = res.tile([P, ND * D], BF)    # m natural:  d1-tile -> [:, t*D:(t+1)*D] = [d1-part, d2]

        # ---------------- DMA issue (order = queue priority) ----------------
        # Inputs are host-packed to SBUF layout, so each loads in a few big
        # contiguous DMAs (issue rate ~0.7us/DMA and ~4 in-flight per queue
        # make many small DMAs the start bottleneck). xT s-chunk-major halves
        # on SP, mn behind them; un in 2 big DMAs on ACT.
        for h in range(4):
            nc.scalar.dma_start(out=un[:, h * 2048:(h + 1) * 2048],
                                in_=u_in[:, h * 2048:(h + 1) * 2048])
        for h in range(4):
            nc.sync.dma_start(out=xt[:, h * 1024:(h + 1) * 1024],
                              in_=xt_in[:, h * 1024:(h + 1) * 1024])
        for c in range(1, 4):
            for h in range(2):
                lo = c * (ND * 512) + h * 2048
                nc.sync.dma_start(out=xt[:, lo: lo + 2048],
                                  in_=xt_in[:, lo: lo + 2048])
        for h in range(2):
            nc.sync.dma_start(out=mn[:, h * 4096:(h + 1) * 4096],
                              in_=m_in[:, h * 4096:(h + 1) * 4096])

        # ---------------- Phase A: Z then F ----------------
        with ExitStack() as pa:
            mmps = pa.enter_context(tc.tile_pool(name="mmps", bufs=8, space="PSUM"))

            def z_j(j):
                zp0 = mmps.tile([P, 512], F32, name="zp0", tag="mm")
                zp1 = mmps.tile([P, 512], F32, name="zp1", tag="mm")
                for d in range(ND):
                    stat = xts(d, j * P, P)
                    mm(zp0[:], stat, un[:, d * D: d * D + 512],
                       start=(d == 0), stop=(d == ND - 1))
                    mm(zp1[:], stat, un[:, d * D + 512: (d + 1) * D],
                       start=(d == 0), stop=(d == ND - 1))
                nc.scalar.copy(zres[:, j * D: j * D + 512], zp0[:])
                nc.scalar.copy(zres[:, j * D + 512: (j + 1) * D], zp1[:])

            for j in range(NS):
                z_j(j)

            # F[d2, i] = sum_d1 m[d1, d2].T xT[d1, i]
            # ic-outer so phase B's superblock sbi=ic never waits on late F
            for ic in range(4):
                for t2 in range(ND):
                    fp = mmps.tile([P, 512], F32, name="fp", tag="mm")
                    for t1 in range(ND):
                        stat = mn[:, t1 * D + t2 * P: t1 * D + (t2 + 1) * P]
                        mm(fp[:], stat, xts(t1, ic * 512, 512),
                           start=(t1 == 0), stop=(t1 == ND - 1))
                    nc.scalar.copy(fres[:, t2 * S + ic * 512: t2 * S + (ic + 1) * 512],
                                   fp[:])

        # ---------------- Phase B ----------------
        with ExitStack() as pb:
            scps = pb.enter_context(tc.tile_pool(name="scps", bufs=3, space="PSUM"))
            outps = pb.enter_context(tc.tile_pool(name="outps", bufs=4, space="PSUM"))
            miscps = pb.enter_context(tc.tile_pool(name="miscps", bufs=1, space="PSUM"))
            expp = pb.enter_context(tc.tile_pool(name="expp", bufs=18))
            taccp = pb.enter_context(tc.tile_pool(name="taccp", bufs=2))
            rbp = pb.enter_context(tc.tile_pool(name="rbp", bufs=2))
            rcp = pb.enter_context(tc.tile_pool(name="rcp", bufs=8))
            outsb = pb.enter_context(tc.tile_pool(name="outsb", bufs=3))

            for sbi in range(NSB):
                # scoresT[j, i] + exp, with DVE rowsum accumulation chasing
                tacc = taccp.tile([P, SB], F32, name="tacc", tag="ta")
                ets = []
                for j in range(NS):
                    sc = scps.tile([P, SB], F32, tag="sc")
                    for t2 in range(ND):
                        mm(sc[:],
                           xts(t2, j * P, P),
                           fres[:, t2 * S + sbi * SB: t2 * S + (sbi + 1) * SB],
                           start=(t2 == 0), stop=(t2 == ND - 1))
                    et = expp.tile([P, SB], BF, name=f"et{j}", tag="et")
                    nc.scalar.activation(et[:], sc[:], EXP)
                    ets.append(et)
                    if j == 0:
                        nc.vector.tensor_copy(tacc[:], et[:])
                    else:
                        nc.vector.tensor_add(tacc[:], tacc[:], et[:])

                # R[i] broadcast across partitions (GPSIMD, off the PE path)
                rbc = rbp.tile([P, SB], F32, name="rbc", tag="rb")
                nc.gpsimd.partition_all_reduce(rbc[:], tacc[:], P,
                                               bass_isa.ReduceOp.add)

                # out[i, do] = sum_j e[j, i-tile].T Z[j, do]
                recips = [None] * NIT
                def out_group(it, ch):
                    op = outps.tile([P, 512], F32, name=f"op{ch}", tag="op")
                    for j in range(NS):
                        mm(op[:], ets[j][:, it * P:(it + 1) * P],
                           zres[:, j * D + ch * 512: j * D + (ch + 1) * 512],
                           start=(j == 0), stop=(j == NS - 1))
                    return op

                def out_evict(it, ch, op):
                    row = (sbi * NIT + it) * P
                    ob = outsb.tile([P, 512], F32, tag="ob")
                    nc.scalar.activation(ob[:], op[:], COPY,
                                         scale=recips[it][:, 0:1])
                    nc.sync.dma_start(
                        out=out_d[row:row + P, ch * 512:(ch + 1) * 512],
                        in_=ob[:])

                # it=0: both mm groups first, then the rc chain (giving the
                # DVE+GPSIMD rowsum ~7us of slack after the last exp), then
                # the two evicts
                op0 = out_group(0, 0)
                op1 = out_group(0, 1)
                for it2 in range(NIT):
                    tp = miscps.tile([P, 1], F32, name=f"rtp{it2}", tag="m")
                    nc.tensor.transpose(tp[:], rbc[0:1, it2 * P:(it2 + 1) * P],
                                        ident_f32[0:1, 0:1])
                    rc = rcp.tile([P, 1], F32, name=f"rc{it2}", tag="rc")
                    nc.vector.reciprocal(rc[:], tp[:])
                    recips[it2] = rc
                out_evict(0, 0, op0)
                out_evict(0, 1, op1)
                last = (sbi == NSB - 1)
                for it in range(1, NIT):
                    for ch in range(2):
                        if last and it == NIT - 1 and ch == 1:
                            continue
                        op = out_group(it, ch)
                        out_evict(it, ch, op)
                if last:
                    # final group: two 256-wide accumulations so the closing
                    # evict+store chain is half as long, stores issued from
                    # the ACT queue (no cross-engine sem before the last DMA)
                    it = NIT - 1
                    row = (sbi * NIT + it) * P
                    op = outps.tile([P, 512], F32, name="opl", tag="op")
                    for h in range(2):
                        for j in range(NS):
                            mm(op[:, h * 256:(h + 1) * 256],
                               ets[j][:, it * P:(it + 1) * P],
                               zres[:, j * D + 512 + h * 256: j * D + 512 + (h + 1) * 256],
                               start=(j == 0), stop=(j == NS - 1))
                        ob = outsb.tile([P, 256], F32, tag="ob")
                        nc.scalar.activation(ob[:], op[:, h * 256:(h + 1) * 256], COPY,
                                             scale=recips[it][:, 0:1])
                        nc.scalar.dma_start(
                            out=out_d[row:row + P, 512 + h * 256: 512 + (h + 1) * 256],
                            in_=ob[:])

    nc.compile()
    return nc


_NC_CACHE = None


def _bf16(a):
    import ml_dtypes
    return np.asarray(a, dtype=np.float32).astype(ml_dtypes.bfloat16)


def _pack_dd(w):
    """[1024, 1024] -> [128, 8*1024] matching the d-tile SBUF layout."""
    return w.reshape(8, 128, 1024).transpose(1, 0, 2).reshape(128, 8192).copy()


def kernel(x, wq, wk, wv, wo):
    global _NC_CACHE
    if _NC_CACHE is None:
        _NC_CACHE = _build()
    nc = _NC_CACHE

    # Host-side weight folding (one-time input transformation) + bf16 casts.
    wq32 = np.asarray(wq, dtype=np.float32)
    wk32 = np.asarray(wk, dtype=np.float32)
    wv32 = np.asarray(wv, dtype=np.float32)
    wo32 = np.asarray(wo, dtype=np.float32)
    m_bf = _pack_dd(_bf16((wq32.T @ wk32) / 32.0))
    ut_bf = _pack_dd(_bf16((wo32 @ wv32).T))
    # xT chunk-major pack: [p, c*4096 + d*512 + s'] = x[c*512+s', d*128+p]
    xt_packed = [
        _bf16(x[b]).T.reshape(8, 128, 4, 512).transpose(1, 2, 0, 3)
        .reshape(128, 16384).copy()
        for b in range(N_CORES)
    ]

    core_ids = list(range(N_CORES))
    in_maps = []
    for b in range(N_CORES):
        in_maps.append({
            "xt": xt_packed[b],
            "m": m_bf,
            "ut": ut_bf,
        })
    res = run_bass_kernel_spmd(nc, in_maps, core_ids)
    return np.stack([res.results[b]["out"] for b in range(N_CORES)], axis=0)
